# revision 16
# baseline (speedup 1.0000x reference)
"""CascadeHadamardSmoothLinear Trainium2 kernel (v3.5).

out = Q_nvfp4(hadamard_rotate(x * smooth_scale * S_in)) @ W.T + bias

Sharding: data-parallel over batch*seq rows across 8 cores; everything else
replicated.  Host pre-computes everything that doesn't need the device:
  - x is pre-scaled by (smooth_scale * S_in) in f32 (exact), sharded,
    transposed to xT [Din, rows] and cast to fp16
  - W is packed to wP [8 groups, 128 kp, 32 kb, 512 n] fp16 so each
    quarter-group load is one big contiguous-line DMA
  - bias is pre-transposed to biasT [128, 32 tiles] f32
  - H_block is cast to fp16

Device pipeline per core (rows=512, Din=Dout=4096), activations fp16.
Prep runs in 16 half-chunks (2 Hadamard blocks each):
  - rotation: rps[m, l] = xT_b.T @ H  (fp16 matmul, f32 PSUM, 8 MMs/half)
  - ACT casts PSUM -> SBUF fp16: rab=|rps|, sgn=sign(rps) (frees PSUM early)
  - NVFP4 snap on DVE in fp16 (magic-number RNE + bit-round on int16);
    scale apply + sign apply on GPSIMD
  - PE transposes quantized tiles to [k, m] fp16 PSUM, ACT copies to SBUF
  - group-0 n-tiles 0,1 accumulate in parallel with prep (PSUM budget:
    2x(rps 2 + qps 1) + acc 2 = 8 banks); n-tiles 2,3 of group 0 plus
    groups 1-7 run after prep as dense back-to-back fp16 matmuls with
    quarter-group weight tiles prefetched on a 6-deep ring
  - bias added via ACT per-partition bias during PSUM->SBUF drain

fp16 quant arithmetic changes which NVFP4 level ~0.1% of borderline elements
snap to; measured rel err vs the f32 reference is ~1.14e-2 (gate: 2e-2).
"""
from contextlib import ExitStack

import numpy as np

_CACHE = {}


def _build(rows, din, dout, repeat=1):
    """Build the per-core Bass program. Same program on all cores (SPMD)."""
    import concourse.bass as bass
    import concourse.tile as tile
    from concourse import bacc, masks, mybir
    from concourse.alu_op_type import AluOpType as ALU

    F32 = mybir.dt.float32
    F16 = mybir.dt.float16
    I16 = mybir.dt.int16
    AX = mybir.AxisListType
    AF = mybir.ActivationFunctionType

    HB = 128                 # hadamard block
    NB = din // HB           # k-blocks (32)
    NM = rows // 128         # m-tiles (4)
    NH = NB // 2             # prep half-chunks of 2 blocks (16)
    NG = dout // 512         # output groups (8)
    NT = 4                   # 128-wide n-tiles per group
    HW = 2 * HB * NM         # free width of one half-chunk tile (1024)
    QB = 8                   # k-blocks per W load tile (quarter group)
    NQ = NB // QB            # W quarter-tiles per group (4)

    M16 = 1536.0             # 1.5 * 2**10: fp16 magic RNE constant
    B4 = 0x4400              # fp16 bits of 4.0
    RND = 0x0100             # half-ulp when keeping 1 mantissa bit (fp16)
    MSK = -0x0200            # 0xFE00: keep sign+exp+top mantissa bit

    nc = bacc.Bacc("TRN2", target_bir_lowering=False, debug=False)
    xT_d = nc.dram_tensor("xT", (din, rows), F16, kind="ExternalInput")
    h_d = nc.dram_tensor("hb", (HB, HB), F16, kind="ExternalInput")
    w_d = nc.dram_tensor("wP", (NG, 128, NB, 512), F16, kind="ExternalInput")
    b_d = nc.dram_tensor("biasT", (128, NG * NT), F32, kind="ExternalInput")
    oT_d = nc.dram_tensor("outT", (dout, rows), F32, kind="ExternalOutput")

    with tile.TileContext(nc) as tc, ExitStack() as ctx:
        cpool = ctx.enter_context(tc.tile_pool(name="const", bufs=1))
        ht = cpool.tile([HB, HB], F16)
        nc.sync.dma_start(ht[:], h_d[:, :])
        ident = cpool.tile([128, 128], F16)
        masks.make_identity(nc, ident[:])
        biasT = cpool.tile([128, NG * NT], F32)
        nc.sync.dma_start(biasT[:], b_d[:, :])
        # per-block quantized-transposed activations: xqt[b] is [k, m] fp16
        xqt = [
            cpool.tile([128, rows], F16, name=f"xqt{b}", tag=f"xqt{b}")
            for b in range(NB)
        ]

        def _emit_body():
            with (
                tc.tile_pool(name="wq", bufs=9) as wpool,
                tc.tile_pool(name="xload", bufs=2) as xpool,
                tc.tile_pool(name="rab", bufs=2) as rabpool,
                tc.tile_pool(name="sgn", bufs=2) as sgnpool,
                tc.tile_pool(name="scl", bufs=2) as sclpool,
                tc.tile_pool(name="qa", bufs=2) as qapool,
                tc.tile_pool(name="qm", bufs=2) as qmpool,
                tc.tile_pool(name="qr", bufs=2) as qrpool,
                tc.tile_pool(name="qx", bufs=2) as qxpool,
                tc.tile_pool(name="qn", bufs=2) as qnpool,
                tc.tile_pool(name="ot", bufs=4) as opool,
            ):
                # ---- W quarter-tile prefetch ring --------------------------
                # prep consumes groups 0 and 1 interleaved, then 2..7
                worder = [(g, q) for q in range(NQ) for g in (0, 1)] + [
                    (g, q) for g in range(2, NG) for q in range(NQ)
                ]
                wq_tiles = {}
                wq_state = {"next": 0}

                def w_issue():
                    if wq_state["next"] >= len(worder):
                        return
                    g, q = worder[wq_state["next"]]
                    wq_state["next"] += 1
                    t = wpool.tile([128, QB, 512], F16, name="wq", tag="wq")
                    nc.sync.dma_start(t[:], w_d[g, :, q * QB : (q + 1) * QB, :])
                    wq_tiles[(g, q)] = t

                def out_group(g, accs, nts):
                    for nt in nts:
                        ot = opool.tile([128, rows], F32, name="ot", tag="ot")
                        nc.scalar.activation(
                            ot[:],
                            accs[nt][:],
                            AF.Identity,
                            bias=biasT[:, g * NT + nt : g * NT + nt + 1],
                            scale=1.0,
                        )
                        nc.sync.dma_start(
                            oT_d[g * 512 + nt * 128 : g * 512 + (nt + 1) * 128, :],
                            ot[:],
                        )

                w_issue()
                w_issue()

                # ---- prep: rotation + quant + transpose + groups 0,1 nt 0,1 -
                with (
                    tc.tile_pool(name="rp", bufs=1, space="PSUM") as rpool,
                    tc.tile_pool(name="qp", bufs=1, space="PSUM") as qpool,
                    tc.tile_pool(name="acc0", bufs=1, space="PSUM") as a0pool,
                ):
                    accs01 = {
                        (g, nt): a0pool.tile(
                            [128, rows], F32, name=f"a{g}_{nt}", tag=f"a{g}_{nt}"
                        )
                        for g in (0, 1)
                        for nt in (0, 1)
                    }
                    xt = None
                    for h in range(NH):
                        c, jh = h // 2, (h % 2) * 2
                        if h % 2 == 0:
                            xt = xpool.tile([128, 4, rows], F16, name="xt", tag="xt")
                            nc.sync.dma_start(
                                xt[:],
                                xT_d[c * 512 : (c + 1) * 512, :].rearrange(
                                    "(j p) m -> p j m", p=128
                                ),
                            )
                        rps = rpool.tile([128, HW], F32, name="rps", tag="rps")
                        qps = qpool.tile([128, HW], F32, name="qps", tag="qps")
                        for mt in range(NM):
                            for j2 in range(2):
                                o = mt * 256 + j2 * 128
                                nc.tensor.matmul(
                                    rps[:, o : o + 128],
                                    xt[:, jh + j2, mt * 128 : (mt + 1) * 128],
                                    ht[:],
                                    start=True,
                                    stop=True,
                                )
                        # PSUM -> SBUF fp16 casts (rps free after these two)
                        rab = rabpool.tile([128, HW], F16, name="rab", tag="rab")
                        nc.scalar.activation(rab[:], rps[:], AF.Abs)
                        sgn = sgnpool.tile([128, HW], F16, name="sgn", tag="sgn")
                        nc.scalar.sign(sgn[:], rps[:])
                        # ---- NVFP4 snap in fp16 (16-groups along free) -----
                        r3 = rab[:].rearrange("p (g s) -> p g s", s=16)
                        amax = sclpool.tile([128, HW // 16], F16, name="amax", tag="amax")
                        nc.vector.tensor_reduce(amax[:], r3, axis=AX.X, op=ALU.max)
                        s12 = sclpool.tile([128, HW // 16], F16, name="s12", tag="s12")
                        nc.vector.tensor_scalar(
                            s12[:], amax[:], 1.0 / 12.0, 6.2e-5,
                            op0=ALU.mult, op1=ALU.max,
                        )
                        inv12 = sclpool.tile([128, HW // 16], F16, name="inv12", tag="inv12")
                        with nc.allow_low_precision(reason="fp16 quant scales"):
                            nc.vector.reciprocal(inv12[:], s12[:])
                        inv_bc = inv12[:].unsqueeze(2).broadcast_to((128, HW // 16, 16))
                        s12_bc = s12[:].unsqueeze(2).broadcast_to((128, HW // 16, 16))

                        ya = qapool.tile([128, HW], F16, name="ya", tag="ya")
                        nc.vector.tensor_tensor(
                            ya[:].rearrange("p (g s) -> p g s", s=16), r3, inv_bc,
                            op=ALU.mult,
                        )
                        # A-branch: e = min(ya, 4.5) + M  (RNE to int via magic)
                        e = qmpool.tile([128, HW], F16, name="e", tag="e")
                        nc.vector.tensor_scalar(
                            e[:], ya[:], 4.5, M16, op0=ALU.min, op1=ALU.add
                        )
                        # B-branch (int16): bitround(max(ya, 4.0)) to {4,6,8,12}
                        g1 = qmpool.tile([128, HW], I16, name="g1", tag="g1")
                        nc.vector.tensor_scalar(
                            g1[:], ya[:].bitcast(I16), B4, RND,
                            op0=ALU.max, op1=ALU.add,
                        )
                        b2 = qmpool.tile([128, HW], I16, name="b2", tag="b2")
                        nc.vector.tensor_scalar(
                            b2[:], g1[:], MSK, None, op0=ALU.bitwise_and
                        )
                        # r2m4 = (e - (M+4)) + b2 == 2*level
                        r2m4 = qrpool.tile([128, HW], F16, name="r2m4", tag="r2m4")
                        nc.vector.scalar_tensor_tensor(
                            r2m4[:], e[:], M16 + 4.0, b2[:].bitcast(F16),
                            op0=ALU.subtract, op1=ALU.add,
                        )
                        # xq = 2*level * (amax/12) * sign   (on GPSIMD)
                        xqm = qxpool.tile([128, HW], F16, name="xqm", tag="xqm")
                        nc.gpsimd.tensor_tensor(
                            xqm[:].rearrange("p (g s) -> p g s", s=16),
                            r2m4[:].rearrange("p (g s) -> p g s", s=16),
                            s12_bc, op=ALU.mult,
                        )
                        xqn = qnpool.tile([128, HW], F16, name="xqn", tag="xqn")
                        nc.gpsimd.tensor_tensor(xqn[:], xqm[:], sgn[:], op=ALU.mult)
                        # transpose quantized tiles into [k, m] via regular
                        # matmul against identity (xqn_s.T @ I) — cheaper than
                        # transpose-mode and counts as PE activity for HAM
                        for j2 in range(2):
                            for mt in range(NM):
                                nc.tensor.matmul(
                                    qps[:, j2 * 512 + mt * 128 : j2 * 512 + (mt + 1) * 128],
                                    xqn[:, mt * 256 + j2 * 128 : mt * 256 + (j2 + 1) * 128],
                                    ident[:],
                                    start=True,
                                    stop=True,
                                )
                        for j2 in range(2):
                            b = 2 * h + j2
                            nc.scalar.copy(
                                xqt[b][:], qps[:, j2 * 512 : (j2 + 1) * 512]
                            )
                        # groups 0,1 accumulation (n-tiles 0,1) for these blocks
                        for j2 in range(2):
                            b = 2 * h + j2
                            for g in (0, 1):
                                wt = wq_tiles[(g, b // QB)]
                                for nt in range(2):
                                    nc.tensor.matmul(
                                        accs01[(g, nt)][:],
                                        wt[:, b % QB, nt * 128 : (nt + 1) * 128],
                                        xqt[b][:],
                                        start=(b == 0),
                                        stop=(b == NB - 1),
                                    )
                        w_issue()
                    out_group(0, [accs01[(0, 0)], accs01[(0, 1)]], nts=(0, 1))
                    out_group(1, [accs01[(1, 0)], accs01[(1, 1)]], nts=(0, 1))

                # ---- groups 0,1 n-tiles 2,3 + groups 2-7: dense matmuls ----
                with tc.tile_pool(name="acc", bufs=1, space="PSUM") as apool:
                    accsA = [
                        apool.tile([128, rows], F32, name=f"aA{t}", tag=f"aA{t}")
                        for t in range(NT)
                    ]
                    accsB = [
                        apool.tile([128, rows], F32, name=f"aB{t}", tag=f"aB{t}")
                        for t in range(NT)
                    ]
                    for g in range(NG):
                        accs = accsA if (g % 2 == 0) else accsB
                        nts = (2, 3) if g in (0, 1) else (0, 1, 2, 3)
                        for q in range(NQ):
                            wt = wq_tiles.pop((g, q))
                            for jb in range(QB):
                                b = q * QB + jb
                                for nt in nts:
                                    nc.tensor.matmul(
                                        accs[nt][:],
                                        wt[:, jb, nt * 128 : (nt + 1) * 128],
                                        xqt[b][:],
                                        start=(b == 0),
                                        stop=(b == NB - 1),
                                    )
                            w_issue()
                        out_group(g, accs, nts)

        for _rep in range(repeat):
            _emit_body()

    nc.compile()
    return nc


def _get_program(rows, din, dout):
    key = (rows, din, dout)
    if key not in _CACHE:
        _CACHE[key] = _build(rows, din, dout)
    return _CACHE[key]


def _prepare_in_maps(x, smooth_scale, S_in, H_block, w_quantized, bias, n_cores):
    """Host-side prep: fold smooth scales into x, shard, pack, cast fp16."""
    B, S, DIN = x.shape
    DOUT = w_quantized.shape[0]
    rows_total = B * S
    rows = rows_total // n_cores

    sv = (np.asarray(smooth_scale, np.float32) * np.asarray(S_in, np.float32))
    xs = np.asarray(x, np.float32).reshape(rows_total, DIN) * sv[None, :]
    hb = np.asarray(H_block, np.float32).astype(np.float16)
    # wP[g, kp, kb, n] = W[g*512+n, kb*128+kp]
    wP = np.ascontiguousarray(
        np.asarray(w_quantized, np.float32)
        .reshape(DOUT // 512, 512, DIN // 128, 128)
        .transpose(0, 3, 2, 1)
    ).astype(np.float16)
    biasT = np.ascontiguousarray(
        np.asarray(bias, np.float32).reshape(DOUT // 128, 128).T
    )
    in_maps = []
    for i in range(n_cores):
        xT = np.ascontiguousarray(xs[i * rows : (i + 1) * rows].T).astype(np.float16)
        in_maps.append({"xT": xT, "hb": hb, "wP": wP, "biasT": biasT})
    return in_maps, rows, DIN, DOUT


def kernel(x, smooth_scale, S_in, H_block, w_quantized, bias):
    from concourse import bass_utils

    B, S, DIN = x.shape
    DOUT = w_quantized.shape[0]
    n_cores = 8
    in_maps, rows, _, _ = _prepare_in_maps(
        x, smooth_scale, S_in, H_block, w_quantized, bias, n_cores
    )
    nc = _get_program(rows, DIN, DOUT)
    res = bass_utils.run_bass_kernel_spmd(nc, in_maps, core_ids=list(range(n_cores)))
    out = np.concatenate([r["outT"].T for r in res.results], axis=0)
    return np.ascontiguousarray(out.reshape(B, S, DOUT).astype(np.float32))


# revision 19
# speedup vs baseline: 1.1915x; 1.1915x over previous
"""CascadeHadamardSmoothLinear Trainium2 kernel (v3.5).

out = Q_nvfp4(hadamard_rotate(x * smooth_scale * S_in)) @ W.T + bias

Sharding: data-parallel over batch*seq rows across 8 cores; everything else
replicated.  Host pre-computes everything that doesn't need the device:
  - x is pre-scaled by (smooth_scale * S_in) in f32 (exact), sharded,
    transposed to xT [Din, rows] and cast to fp16
  - W is packed to wP [8 groups, 128 kp, 32 kb, 512 n] fp16 so each
    quarter-group load is one big contiguous-line DMA
  - bias is pre-transposed to biasT [128, 32 tiles] f32
  - H_block is cast to fp16

Device pipeline per core (rows=512, Din=Dout=4096), activations fp16.
Prep runs in 16 half-chunks (2 Hadamard blocks each):
  - rotation: rps[m, l] = xT_b.T @ H  (fp16 matmul, f32 PSUM, 8 MMs/half)
  - ACT casts PSUM -> SBUF fp16: rab=|rps|, sgn=sign(rps) (frees PSUM early)
  - NVFP4 snap on DVE in fp16 (magic-number RNE + bit-round on int16);
    scale apply + sign apply on GPSIMD
  - PE transposes quantized tiles to [k, m] fp16 PSUM, ACT copies to SBUF
  - group-0 n-tiles 0,1 accumulate in parallel with prep (PSUM budget:
    2x(rps 2 + qps 1) + acc 2 = 8 banks); n-tiles 2,3 of group 0 plus
    groups 1-7 run after prep as dense back-to-back fp16 matmuls with
    quarter-group weight tiles prefetched on a 6-deep ring
  - bias added via ACT per-partition bias during PSUM->SBUF drain

fp16 quant arithmetic changes which NVFP4 level ~0.1% of borderline elements
snap to; measured rel err vs the f32 reference is ~1.14e-2 (gate: 2e-2).
"""
from contextlib import ExitStack

import numpy as np

_CACHE = {}


def _build(rows, din, dout, repeat=1):
    """Build the per-core Bass program. Same program on all cores (SPMD)."""
    import concourse.bass as bass
    import concourse.tile as tile
    from concourse import bacc, masks, mybir
    from concourse.alu_op_type import AluOpType as ALU

    F32 = mybir.dt.float32
    F16 = mybir.dt.float16
    I16 = mybir.dt.int16
    AX = mybir.AxisListType
    AF = mybir.ActivationFunctionType

    HB = 128                 # hadamard block
    NB = din // HB           # k-blocks (32)
    NM = rows // 128         # m-tiles (4)
    NH = NB // 2             # prep half-chunks of 2 blocks (16)
    NG = dout // 512         # output groups (8)
    NT = 4                   # 128-wide n-tiles per group
    HW = 2 * HB * NM         # free width of one half-chunk tile (1024)
    QB = 8                   # k-blocks per W load tile (quarter group)
    NQ = NB // QB            # W quarter-tiles per group (4)

    M16 = 1536.0             # 1.5 * 2**10: fp16 magic RNE constant
    B4 = 0x4400              # fp16 bits of 4.0
    RND = 0x0100             # half-ulp when keeping 1 mantissa bit (fp16)
    MSK = -0x0200            # 0xFE00: keep sign+exp+top mantissa bit

    nc = bacc.Bacc("TRN2", target_bir_lowering=False, debug=False)
    xT_d = nc.dram_tensor("xT", (din, rows), F16, kind="ExternalInput")
    h_d = nc.dram_tensor("hb", (HB, HB), F16, kind="ExternalInput")
    w_d = nc.dram_tensor("wP", (NG, 128, NB, 512), F16, kind="ExternalInput")
    b_d = nc.dram_tensor("biasT", (128, NG * NT), F32, kind="ExternalInput")
    oT_d = nc.dram_tensor("outT", (dout, rows), F32, kind="ExternalOutput")

    with tile.TileContext(nc) as tc, ExitStack() as ctx:
        cpool = ctx.enter_context(tc.tile_pool(name="const", bufs=1))
        ht = cpool.tile([HB, HB], F16)
        nc.sync.dma_start(ht[:], h_d[:, :])
        ident = cpool.tile([128, 128], F16)
        masks.make_identity(nc, ident[:])
        biasT = cpool.tile([128, NG * NT], F32)
        nc.sync.dma_start(biasT[:], b_d[:, :])
        # per-block quantized-transposed activations: xqt[b] is [k, m] fp16
        xqt = [
            cpool.tile([128, rows], F16, name=f"xqt{b}", tag=f"xqt{b}")
            for b in range(NB)
        ]

        def _emit_body():
            with (
                tc.tile_pool(name="wq", bufs=9) as wpool,
                tc.tile_pool(name="xload", bufs=2) as xpool,
                tc.tile_pool(name="rab", bufs=2) as rabpool,
                tc.tile_pool(name="sgn", bufs=2) as sgnpool,
                tc.tile_pool(name="scl", bufs=2) as sclpool,
                tc.tile_pool(name="qa", bufs=2) as qapool,
                tc.tile_pool(name="qm", bufs=2) as qmpool,
                tc.tile_pool(name="qr", bufs=2) as qrpool,
                tc.tile_pool(name="qx", bufs=2) as qxpool,
                tc.tile_pool(name="qn", bufs=2) as qnpool,
                tc.tile_pool(name="ot", bufs=4) as opool,
            ):
                # ---- W quarter-tile prefetch ring --------------------------
                # prep consumes groups 0 and 1 interleaved, then 2..7
                worder = [(g, q) for q in range(NQ) for g in (0, 1)] + [
                    (g, q) for g in range(2, NG) for q in range(NQ)
                ]
                wq_tiles = {}
                wq_state = {"next": 0}

                def w_issue():
                    if wq_state["next"] >= len(worder):
                        return
                    g, q = worder[wq_state["next"]]
                    wq_state["next"] += 1
                    t = wpool.tile([128, QB, 512], F16, name="wq", tag="wq")
                    nc.sync.dma_start(t[:], w_d[g, :, q * QB : (q + 1) * QB, :])
                    wq_tiles[(g, q)] = t

                def out_group(g, accs, nts):
                    for nt in nts:
                        ot = opool.tile([128, rows], F32, name="ot", tag="ot")
                        nc.scalar.activation(
                            ot[:],
                            accs[nt][:],
                            AF.Identity,
                            bias=biasT[:, g * NT + nt : g * NT + nt + 1],
                            scale=1.0,
                        )
                        nc.sync.dma_start(
                            oT_d[g * 512 + nt * 128 : g * 512 + (nt + 1) * 128, :],
                            ot[:],
                        )

                w_issue()
                w_issue()

                # ---- prep: rotation + quant + transpose + groups 0,1 nt 0,1 -
                with (
                    tc.tile_pool(name="rp", bufs=1, space="PSUM") as rpool,
                    tc.tile_pool(name="qp", bufs=1, space="PSUM") as qpool,
                    tc.tile_pool(name="acc0", bufs=1, space="PSUM") as a0pool,
                ):
                    accs01 = {
                        (g, nt): a0pool.tile(
                            [128, rows], F32, name=f"a{g}_{nt}", tag=f"a{g}_{nt}"
                        )
                        for g in (0, 1)
                        for nt in (0, 1)
                    }
                    xt = None
                    pending = None
                    for h in range(NH):
                        c, jh = h // 2, (h % 2) * 2
                        if h % 2 == 0:
                            xt = xpool.tile([128, 4, rows], F16, name="xt", tag="xt")
                            nc.sync.dma_start(
                                xt[:],
                                xT_d[c * 512 : (c + 1) * 512, :].rearrange(
                                    "(j p) m -> p j m", p=128
                                ),
                            )
                        rps = rpool.tile([128, HW], F32, name="rps", tag="rps")
                        for mt in range(NM):
                            for j2 in range(2):
                                o = mt * 256 + j2 * 128
                                nc.tensor.matmul(
                                    rps[:, o : o + 128],
                                    xt[:, jh + j2, mt * 128 : (mt + 1) * 128],
                                    ht[:],
                                    start=True,
                                    stop=True,
                                )
                        # PSUM -> SBUF fp16 casts (rps free after these two)
                        rab = rabpool.tile([128, HW], F16, name="rab", tag="rab")
                        nc.scalar.activation(rab[:], rps[:], AF.Abs)
                        sgn = sgnpool.tile([128, HW], F16, name="sgn", tag="sgn")
                        nc.scalar.sign(sgn[:], rps[:])
                        # ---- NVFP4 snap in fp16 (16-groups along free) -----
                        r3 = rab[:].rearrange("p (g s) -> p g s", s=16)
                        amax = sclpool.tile([128, HW // 16], F16, name="amax", tag="amax")
                        nc.vector.tensor_reduce(amax[:], r3, axis=AX.X, op=ALU.max)
                        s12 = sclpool.tile([128, HW // 16], F16, name="s12", tag="s12")
                        nc.vector.tensor_scalar(
                            s12[:], amax[:], 1.0 / 12.0, 6.2e-5,
                            op0=ALU.mult, op1=ALU.max,
                        )
                        inv12 = sclpool.tile([128, HW // 16], F16, name="inv12", tag="inv12")
                        with nc.allow_low_precision(reason="fp16 quant scales"):
                            nc.vector.reciprocal(inv12[:], s12[:])
                        inv_bc = inv12[:].unsqueeze(2).broadcast_to((128, HW // 16, 16))
                        s12_bc = s12[:].unsqueeze(2).broadcast_to((128, HW // 16, 16))

                        ya = qapool.tile([128, HW], F16, name="ya", tag="ya")
                        nc.vector.tensor_tensor(
                            ya[:].rearrange("p (g s) -> p g s", s=16), r3, inv_bc,
                            op=ALU.mult,
                        )
                        # A-branch: e = min(ya, 4.5) + M  (RNE to int via magic)
                        e = qmpool.tile([128, HW], F16, name="e", tag="e")
                        nc.vector.tensor_scalar(
                            e[:], ya[:], 4.5, M16, op0=ALU.min, op1=ALU.add
                        )
                        # B-branch (int16): bitround(max(ya, 4.0)) to {4,6,8,12}
                        g1 = qmpool.tile([128, HW], I16, name="g1", tag="g1")
                        nc.vector.tensor_scalar(
                            g1[:], ya[:].bitcast(I16), B4, RND,
                            op0=ALU.max, op1=ALU.add,
                        )
                        b2 = qmpool.tile([128, HW], I16, name="b2", tag="b2")
                        nc.vector.tensor_scalar(
                            b2[:], g1[:], MSK, None, op0=ALU.bitwise_and
                        )
                        # r2m4 = (e - (M+4)) + b2 == 2*level
                        r2m4 = qrpool.tile([128, HW], F16, name="r2m4", tag="r2m4")
                        nc.vector.scalar_tensor_tensor(
                            r2m4[:], e[:], M16 + 4.0, b2[:].bitcast(F16),
                            op0=ALU.subtract, op1=ALU.add,
                        )
                        # xq = 2*level * (amax/12) * sign   (on GPSIMD)
                        xqm = qxpool.tile([128, HW], F16, name="xqm", tag="xqm")
                        nc.gpsimd.tensor_tensor(
                            xqm[:].rearrange("p (g s) -> p g s", s=16),
                            r2m4[:].rearrange("p (g s) -> p g s", s=16),
                            s12_bc, op=ALU.mult,
                        )
                        xqn = qnpool.tile([128, HW], F16, name="xqn", tag="xqn")
                        nc.gpsimd.tensor_tensor(xqn[:], xqm[:], sgn[:], op=ALU.mult)

                        # PE follow-up (transposes + group-0/1 MMs) is emitted
                        # one half LATE: the PE engine queue is in-order, so
                        # half h's transposes (which wait on the quant chain)
                        # must not sit ahead of work that is already runnable.
                        def followup(hh, xqn_t):
                            qps = qpool.tile([128, HW], F32, name="qps", tag="qps")
                            # transpose via regular matmul against identity
                            # (xqn.T @ I) — pipelines like a matmul and counts
                            # as PE activity for the HAM clock gate
                            for j2 in range(2):
                                for mt in range(NM):
                                    nc.tensor.matmul(
                                        qps[:, j2 * 512 + mt * 128 : j2 * 512 + (mt + 1) * 128],
                                        xqn_t[:, mt * 256 + j2 * 128 : mt * 256 + (j2 + 1) * 128],
                                        ident[:],
                                        start=True,
                                        stop=True,
                                    )
                            for j2 in range(2):
                                b = 2 * hh + j2
                                nc.scalar.copy(
                                    xqt[b][:], qps[:, j2 * 512 : (j2 + 1) * 512]
                                )
                            for j2 in range(2):
                                b = 2 * hh + j2
                                for g in (0, 1):
                                    wt = wq_tiles[(g, b // QB)]
                                    for nt in range(2):
                                        nc.tensor.matmul(
                                            accs01[(g, nt)][:],
                                            wt[:, b % QB, nt * 128 : (nt + 1) * 128],
                                            xqt[b][:],
                                            start=(b == 0),
                                            stop=(b == NB - 1),
                                        )
                            w_issue()

                        if pending is not None:
                            followup(*pending)
                        pending = (h, xqn)
                    followup(*pending)
                    out_group(0, [accs01[(0, 0)], accs01[(0, 1)]], nts=(0, 1))
                    out_group(1, [accs01[(1, 0)], accs01[(1, 1)]], nts=(0, 1))

                # ---- groups 0,1 n-tiles 2,3 + groups 2-7: dense matmuls ----
                with tc.tile_pool(name="acc", bufs=1, space="PSUM") as apool:
                    accsA = [
                        apool.tile([128, rows], F32, name=f"aA{t}", tag=f"aA{t}")
                        for t in range(NT)
                    ]
                    accsB = [
                        apool.tile([128, rows], F32, name=f"aB{t}", tag=f"aB{t}")
                        for t in range(NT)
                    ]
                    for g in range(NG):
                        accs = accsA if (g % 2 == 0) else accsB
                        nts = (2, 3) if g in (0, 1) else (0, 1, 2, 3)
                        for q in range(NQ):
                            wt = wq_tiles.pop((g, q))
                            for jb in range(QB):
                                b = q * QB + jb
                                for nt in nts:
                                    nc.tensor.matmul(
                                        accs[nt][:],
                                        wt[:, jb, nt * 128 : (nt + 1) * 128],
                                        xqt[b][:],
                                        start=(b == 0),
                                        stop=(b == NB - 1),
                                    )
                            w_issue()
                        out_group(g, accs, nts)

        for _rep in range(repeat):
            _emit_body()

    nc.compile()
    return nc


def _get_program(rows, din, dout):
    key = (rows, din, dout)
    if key not in _CACHE:
        _CACHE[key] = _build(rows, din, dout)
    return _CACHE[key]


def _prepare_in_maps(x, smooth_scale, S_in, H_block, w_quantized, bias, n_cores):
    """Host-side prep: fold smooth scales into x, shard, pack, cast fp16."""
    B, S, DIN = x.shape
    DOUT = w_quantized.shape[0]
    rows_total = B * S
    rows = rows_total // n_cores

    sv = (np.asarray(smooth_scale, np.float32) * np.asarray(S_in, np.float32))
    xs = np.asarray(x, np.float32).reshape(rows_total, DIN) * sv[None, :]
    hb = np.asarray(H_block, np.float32).astype(np.float16)
    # wP[g, kp, kb, n] = W[g*512+n, kb*128+kp]
    wP = np.ascontiguousarray(
        np.asarray(w_quantized, np.float32)
        .reshape(DOUT // 512, 512, DIN // 128, 128)
        .transpose(0, 3, 2, 1)
    ).astype(np.float16)
    biasT = np.ascontiguousarray(
        np.asarray(bias, np.float32).reshape(DOUT // 128, 128).T
    )
    in_maps = []
    for i in range(n_cores):
        xT = np.ascontiguousarray(xs[i * rows : (i + 1) * rows].T).astype(np.float16)
        in_maps.append({"xT": xT, "hb": hb, "wP": wP, "biasT": biasT})
    return in_maps, rows, DIN, DOUT


def kernel(x, smooth_scale, S_in, H_block, w_quantized, bias):
    from concourse import bass_utils

    B, S, DIN = x.shape
    DOUT = w_quantized.shape[0]
    n_cores = 8
    in_maps, rows, _, _ = _prepare_in_maps(
        x, smooth_scale, S_in, H_block, w_quantized, bias, n_cores
    )
    nc = _get_program(rows, DIN, DOUT)
    res = bass_utils.run_bass_kernel_spmd(nc, in_maps, core_ids=list(range(n_cores)))
    out = np.concatenate([r["outT"].T for r in res.results], axis=0)
    return np.ascontiguousarray(out.reshape(B, S, DOUT).astype(np.float32))
